# revision 60
# baseline (speedup 1.0000x reference)
"""Trainium2 Bass kernel for a GQA attention layer (dense transformer).

Reference computation (B=1, S=2048, DIM=2048, 32 q-heads, 8 kv-heads, hd=64):
    xq = x @ wq; xk = x @ wk; xv = x @ wv
    rope(xq, xk); GQA causal attention; out = attn @ wo

Sharding: tensor-parallel over heads across 8 cores. Core c owns q-heads
4c..4c+3 (wq cols), kv-head c (wk/wv cols), and wo rows 256c..256c+255.
Each core computes a full [S, DIM] partial of the output projection; the
host sums the 8 partials (the TP all-reduce, done at gather time).

Kernel layout strategy (everything "transposed", head_dim on partitions):
  - QT/KT/VT = W.T @ x computed with lhsT = weight shard (natural [DIM, m]
    layout), rhs = x.T tiles streamed from DRAM.
  - RoPE pairs are de-interleaved by permuting wq/wk columns on the host so
    the rotation partner sits 16 partitions away (within a 32-partition
    quadrant, so DVE stream_shuffle can swap them).
  - Scores are computed transposed: S^T[k, q] = K^T.T @ Q^T per 128-row
    k-tile; exp on ACT (scale fused); causal mask = upper-tri multiply on
    the single diagonal 128x128 block of each k-tile.
  - Projections run as split-fp8 DoubleRow matmuls: x and wqkv are split
    host-side into fp8e4m3 hi+lo pairs at one power-of-2 scale; the three
    series (hh, hl, lh) accumulate in one PSUM group at 0.5 cycles/row and
    ~0.03% error.  Descale is folded into the exp scale and the V copy-out.
  - P@V is "Form B": per 128-q-tile accumulators po[128q, 65] = P^T.T @ V'
    (V' = [V | ones], col 64 = softmax denominator), four q-tiles packed in
    one PSUM bank (qt3's first matmul carries start=True, pending-zeroing
    the bank for its siblings, which accumulate with start=False).
    Normalization
    is then a per-partition reciprocal + tensor_scalar mult; O is
    transposed back to the d-major OT layout on the PE.
  - Output projection from O^T with wo shard as rhs; partial DMA'd f16 and
    summed across cores on the host (the TP all-reduce).
"""

import numpy as np
import ml_dtypes

import concourse.bass as bass
import concourse.mybir as mybir
from concourse import bacc
from concourse.tile import TileContext
from concourse.masks import make_identity
from concourse.bass_utils import run_bass_kernel_spmd

# ---------------------------------------------------------------- constants
S = 2048          # sequence length
DIM = 2048        # model dim
NH = 32           # query heads
NKV = 8           # kv heads
HD = 64           # head dim
NCORES = 8
HQ = NH // NCORES          # query heads per core = 4
QW = HQ * HD               # q width per core = 256
KT_S = S // 128            # 16 seq k-tiles
KT_D = DIM // 128          # 16 dim k-tiles
NSC = S // 512             # 4 s-chunks
SCALE = 1.0 / 8.0          # 1/sqrt(64)

# split-fp8 projection scales: x ~ (hi + lo)/SXH, w ~ (hi + lo)/SWH with both
# residuals stored at the SAME power-of-2 scale, so the three DoubleRow
# series (hh, hl, lh) accumulate into one PSUM group; 1/(SXH*SWH) is folded
# into the rope tables (Q, K) and the V copy-out.
SXH = 8.0
SWH = 512.0
PROJ_DESCALE = 1.0 / (SXH * SWH)
# rope tables are plain (descale folded into exp scale / V copy instead), so
# QT/KVt hold rope values at the raw x*w scale (max ~|q|*4096 < f16 range)
EXP_SCALE = SCALE * PROJ_DESCALE * PROJ_DESCALE
# WO also runs as split-fp8 DoubleRow: OT is rescaled (x32, fits e4m3) and
# split into hi+lo on the otherwise-idle GPSIMD; wo is split host-side
SO = 32.0
SWO = 512.0
WO_DESCALE = 1.0 / (SO * SWO)

_SHUF_SWAP16 = list(range(16, 32)) + list(range(16))


def _dtypes():
    return mybir.dt.float16, mybir.dt.float16, np.float16


def _mm_ap(ap, mmdt):
    """View an AP in the matmul dtype (bitcast f32 -> f32r when needed)."""
    if ap.dtype != mmdt:
        return ap.bitcast(mmdt)
    return ap


def build_program():
    """Build the per-core Bass program (same program on all 8 cores).

    Emission is a fine-grained software pipeline: attention beats for chunk
    sc (S^T mega-matmul for head h + PV pairs of head h-1) are merged with
    the projection matmuls of chunk sc+1 and the WO units of chunk sc-1.
    Engines execute in order, so the merge keeps only likely-ready work in
    the PE stream while ACT (exp) and DVE (RoPE/normalize) drain.

    PSUM (8 banks): pjo 3 (projection passes + PV accumulators, shared tag)
    + ps 4 (two [128,1024] score megas) + pw 1 (WO).
    """
    sdt, mmdt, _ = _dtypes()
    f32 = mybir.dt.float32
    f8 = mybir.dt.float8e4
    DR = mybir.MatmulPerfMode.DoubleRow

    nc = bacc.Bacc("TRN2", target_bir_lowering=False, debug=False,
                   num_devices=NCORES)

    x8h = nc.dram_tensor("x8h", [DIM, S], f8, kind="ExternalInput")
    x8l = nc.dram_tensor("x8l", [DIM, S], f8, kind="ExternalInput")
    w8h = nc.dram_tensor("w8h", [DIM, QW + 2 * HD], f8,
                         kind="ExternalInput")
    w8l = nc.dram_tensor("w8l", [DIM, QW + 2 * HD], f8,
                         kind="ExternalInput")
    wo8h = nc.dram_tensor("wo8h", [QW, DIM], f8, kind="ExternalInput")
    wo8l = nc.dram_tensor("wo8l", [QW, DIM], f8, kind="ExternalInput")
    cosE = nc.dram_tensor("cosE", [64, S], sdt, kind="ExternalInput")
    sinE = nc.dram_tensor("sinE", [64, S], sdt, kind="ExternalInput")
    utri = nc.dram_tensor("utri", [128, 128], sdt, kind="ExternalInput")
    out = nc.dram_tensor("out", [S, DIM], sdt, kind="ExternalOutput")

    WQKV = QW + 2 * HD  # 384

    import contextlib
    with TileContext(nc) as tc, contextlib.ExitStack() as ctx:
        const = ctx.enter_context(tc.tile_pool(name="const", bufs=1))
        work = ctx.enter_context(tc.tile_pool(name="work", bufs=2))
        xtp = ctx.enter_context(tc.tile_pool(name="xtp", bufs=16))
        ptp = ctx.enter_context(tc.tile_pool(name="ptp", bufs=20))
        small = ctx.enter_context(tc.tile_pool(name="small", bufs=5))
        osb = ctx.enter_context(tc.tile_pool(name="osb", bufs=4))

        pjo = ctx.enter_context(tc.tile_pool(name="pjo", bufs=2,
                                             space="PSUM"))
        ps = ctx.enter_context(tc.tile_pool(name="ps", bufs=2, space="PSUM"))
        pw = ctx.enter_context(tc.tile_pool(name="pw", bufs=1, space="PSUM"))
        otq = ctx.enter_context(tc.tile_pool(name="otq", bufs=6))

        # ----------------------------------------------- persistent SBUF
        w_sbh = const.tile([128, KT_D, WQKV], f8, tag="w_sbh")
        w_sbl = const.tile([128, KT_D, WQKV], f8, tag="w_sbl")
        wo_sbh = const.tile([128, 2, DIM], f8, tag="wo_sbh")
        wo_sbl = const.tile([128, 2, DIM], f8, tag="wo_sbl")
        cos_sb = const.tile([128, S], sdt, tag="cos_sb")
        sin_sb = const.tile([128, S], sdt, tag="sin_sb")
        utri_sb = const.tile([128, 128], sdt, tag="utri_sb")
        ident = const.tile([128, 128], sdt, tag="ident")
        QT = const.tile([64, HQ * S], sdt, tag="QT")
        KVt = const.tile([128, S], sdt, tag="KVt")
        Vp = const.tile([128, KT_S * (HD + 1)], sdt, tag="Vp")
        OT = const.tile([128, 2 * S], sdt, tag="OT")
        OT8h = const.tile([128, 2, S], f8, tag="OT8h")
        OT8l = const.tile([128, 2, S], f8, tag="OT8l")

        make_identity(nc, ident[:])
        nc.gpsimd.memset(Vp[:], 1.0 / SO)  # ones cols scaled: denom = sum(P)/SO,
        # so normalize yields SO*o directly (pre-scaled for the fp8 split)
        zot = const.tile([128, 128], sdt, tag="zot")
        nc.gpsimd.memset(zot[:], 0.0)

        wo_copy_flip = [0]
        o_sbs = {}                       # (qc, qtl) -> [128, 256] O tile

        # ---------------------------------------------- thunk generators
        def proj_thunks(sc, fused=False):
            """Projection of chunk sc via split-fp8 DoubleRow: three matmul
            series (x_hi*w_hi, x_hi*w_lo, x_lo*w_hi) per k-tile PAIR
            accumulate into one PSUM group at a shared power-of-2 scale.
            KV pass, K-rope, V transposes, then Q passes. With fused=True
            (prologue) all three output groups run per pair, using 3 slots."""
            s0 = sc * 512
            xts = []
            st = {}

            batches = [2, 2, 4, 4, 4] if sc == 0 else [4, 4, 4, 4]
            starts = [sum(batches[:i]) for i in range(len(batches))]
            kt_slot = {}
            for bi, (b0, bn) in enumerate(zip(starts, batches)):
                for j in range(bn):
                    kt_slot[b0 + j] = (bi, j, bn, b0)

            def pair_mms(psum, c0, cm, t):
                """The 3 DoubleRow series for k-pair t into psum[:, 0:512]."""
                bi, j, bn, b0 = kt_slot[2 * t]
                xth, xtl = xts[bi]
                xh = xth[:, j:j + 2, :]
                xl = xtl[:, j:j + 2, :]
                wh = w_sbh[:, 2 * t:2 * t + 2, c0:c0 + cm]
                wl = w_sbl[:, 2 * t:2 * t + 2, c0:c0 + cm]
                for si, (wp, xp) in enumerate(
                        [(wh, xh), (wh, xl), (wl, xh)]):
                    nc.tensor.matmul(
                        psum[:], wp, xp,
                        start=(t == 0 and si == 0),
                        stop=(t == KT_D // 2 - 1 and si == 2),
                        perf_mode=DR)

            def dma_kv(kt):
                bi, j, bn, b0 = kt_slot[kt]
                if j == 0:
                    # queue order w8h, x8h, x8l, w8l: the hh series (emitted
                    # first) waits only the first two transfers
                    if sc == 0:
                        nc.sync.dma_start(
                            w_sbh[:, b0:b0 + bn, :],
                            w8h[b0 * 128:(b0 + bn) * 128, :].rearrange(
                                "(k r) w -> r k w", k=bn))
                    xth = xtp.tile([128, 4, 512], f8, tag="xt", name="xth")
                    xtl = xtp.tile([128, 4, 512], f8, tag="xt", name="xtl")
                    for dst, src in ((xth, x8h), (xtl, x8l)):
                        nc.sync.dma_start(
                            dst[:, 0:bn, :],
                            src[b0 * 128:(b0 + bn) * 128,
                                s0:s0 + 512].rearrange("(k r) c -> r k c",
                                                       k=bn))
                    xts.append((xth, xtl))
                    if sc == 0:
                        nc.sync.dma_start(
                            w_sbl[:, b0:b0 + bn, :],
                            w8l[b0 * 128:(b0 + bn) * 128, :].rearrange(
                                "(k r) w -> r k w", k=bn))
                    if sc == 0 and kt == 4:
                        # constants ride behind the first weight/x batches
                        # but land before the first RoPE needs them
                        # (cos/sin have 64-row periodicity: rows 64:128 are
                        # filled by a cheap on-chip copy, not a second DMA)
                        nc.sync.dma_start(cos_sb[0:64, :], cosE[:])
                        nc.sync.dma_start(sin_sb[0:64, :], sinE[:])
                        nc.sync.dma_start(utri_sb[:], utri[:])
                        nc.vector.tensor_copy(cos_sb[64:128, :],
                                              cos_sb[0:64, :])
                        nc.vector.tensor_copy(sin_sb[64:128, :],
                                              sin_sb[0:64, :])
                if kt % 2 == 0:
                    return
                t = kt // 2
                if t == 0:
                    st["pkv"] = pjo.tile([128, 512], f32, tag="pjo",
                                         name="pkv")
                    if fused:
                        st["fq0"] = pjo.tile([128, 512], f32, tag="pjo",
                                             name="fq0")
                        st["fq1"] = pjo.tile([128, 512], f32, tag="pjo",
                                             name="fq1")
                pair_mms(st["pkv"], 256, 128, t)
                if fused:
                    for mt in range(2):
                        pair_mms(st[f"fq{mt}"], mt * 128, 128, t)

            def k_rope():
                # one f32->f16 copy out of PSUM, then all rope math runs in
                # fp16 SBUF where DVE gets the 2x perf mode
                pkv = st["pkv"]
                kv16 = work.tile([128, 512], sdt, tag="kv16", name="kv16")
                shufk = work.tile([64, 512], sdt, tag="shufk", name="shufk")
                m1k = work.tile([64, 512], sdt, tag="m1k", name="m1k")
                t2k = work.tile([64, 512], sdt, tag="t2k", name="t2k")
                nc.vector.tensor_copy(kv16[:], pkv[:])
                nc.vector.stream_shuffle(shufk[:], kv16[0:64, :],
                                         _SHUF_SWAP16)
                nc.vector.tensor_mul(m1k[:], kv16[0:64, :],
                                     cos_sb[0:64, s0:s0 + 512])
                nc.vector.tensor_mul(t2k[:], shufk[:],
                                     sin_sb[0:64, s0:s0 + 512])
                nc.vector.tensor_add(KVt[0:64, s0:s0 + 512], m1k[:], t2k[:])
                nc.vector.tensor_scalar_mul(KVt[64:128, s0:s0 + 512],
                                            kv16[64:128, :], PROJ_DESCALE)

            def v_trans(kt):
                pv = pw.tile([128, HD], sdt, tag="pw", name="pv")
                nc.tensor.transpose(
                    pv[:], KVt[64:128, kt * 128:(kt + 1) * 128],
                    ident[64:128, 64:128])
                nc.vector.tensor_copy(
                    Vp[:, kt * (HD + 1):kt * (HD + 1) + HD], pv[:])

            def q_mm(mt, t):
                if t == 0:
                    st["pq"] = pjo.tile([128, 512], f32, tag="pjo",
                                        name="pq")
                pair_mms(st["pq"], mt * 128, 128, t)

            def q_rope(mt):
                pq = st[f"fq{mt}"] if fused else st["pq"]
                q16 = work.tile([128, 512], sdt, tag="q16", name="q16")
                shuf = work.tile([128, 512], sdt, tag="shuf", name="shuf")
                m1 = work.tile([128, 512], sdt, tag="m1", name="m1")
                t2 = work.tile([128, 512], sdt, tag="t2", name="t2")
                nc.vector.tensor_copy(q16[:], pq[:])
                nc.vector.stream_shuffle(shuf[:], q16[:], _SHUF_SWAP16)
                nc.vector.tensor_mul(m1[:], q16[:], cos_sb[:, s0:s0 + 512])
                nc.vector.tensor_mul(t2[:], shuf[:], sin_sb[:, s0:s0 + 512])
                he = (2 * mt) * S
                ho = (2 * mt + 1) * S
                nc.vector.tensor_add(
                    QT[:, he + s0:he + s0 + 512], m1[0:64, :], t2[0:64, :])
                nc.vector.tensor_add(
                    QT[:, ho + s0:ho + s0 + 512], m1[64:128, :],
                    t2[64:128, :])

            th = [lambda kt=kt: dma_kv(kt) for kt in range(KT_D)]
            th.append(k_rope)
            if fused:
                th += [lambda kt=kt: v_trans(kt)
                       for kt in range(4 * sc, 4 * sc + 4)]
                th += [lambda mt=mt: q_rope(mt) for mt in range(2)]
            else:
                for mt in range(2):
                    th += [lambda mt=mt, t=t: q_mm(mt, t)
                           for t in range(KT_D // 2)]
                    th.append(lambda mt=mt: q_rope(mt))
                th += [lambda kt=kt: v_trans(kt)
                       for kt in range(4 * sc, 4 * sc + 4)]
            return th

        def s_thunks(qc, h, tiles):
            """S^T mega matmuls + exp + mask for one head; fills `tiles`."""
            q0 = qc * 512
            hf = h * S
            nkt = 4 * qc + 4
            thunks = []
            for pi in range(nkt // 2):
                def th(pi=pi):
                    kts = (2 * pi, 2 * pi + 1)
                    ps_t = ps.tile([128, 1024], f32, tag="ps", name="ps_t")
                    pt_t = ptp.tile([128, 1024], sdt, tag="pt", name="pt_t")
                    for li, kt in enumerate(kts):
                        dj = kt - 4 * qc
                        qo = 128 * dj if dj >= 0 else 0
                        lo = li * 512
                        nc.tensor.matmul(
                            ps_t[:, lo + qo:lo + 512],
                            _mm_ap(KVt[0:64, kt * 128:(kt + 1) * 128], mmdt),
                            _mm_ap(QT[:, hf + q0 + qo:hf + q0 + 512], mmdt),
                            start=True, stop=True)
                    if 2 * pi + 1 < 4 * qc:
                        nc.scalar.activation(
                            pt_t[:], ps_t[:],
                            mybir.ActivationFunctionType.Exp, scale=EXP_SCALE)
                    else:
                        for li, kt in enumerate(kts):
                            dj = kt - 4 * qc
                            qo = 128 * dj if dj >= 0 else 0
                            lo = li * 512
                            nc.scalar.activation(
                                pt_t[:, lo + qo:lo + 512],
                                ps_t[:, lo + qo:lo + 512],
                                mybir.ActivationFunctionType.Exp,
                                scale=EXP_SCALE)
                    for li, kt in enumerate(kts):
                        dj = kt - 4 * qc
                        qo = 128 * dj if dj >= 0 else 0
                        lo = li * 512
                        if dj >= 0:
                            # causal mask, alternating DVE/GPSIMD to keep
                            # both queues short
                            eng = nc.vector if dj % 2 else nc.gpsimd
                            eng.tensor_mul(
                                pt_t[:, lo + qo:lo + qo + 128],
                                pt_t[:, lo + qo:lo + qo + 128], utri_sb[:])
                        tiles.append((kt, qo, lo, pt_t))
                thunks.append(th)
            return thunks

        def pv_thunks(qc, h, tiles):
            """Form-B PV: q-major accumulators po[128q, 65] per q-tile, all
            four packed into one PSUM bank.  The bank is opened by a single
            start=True zero matmul (pending-zero covers the whole bank), the
            real accumulation runs with start=False; column 64 of each slot
            is the softmax denominator (ones column of V').  After the
            diagonal pair: per-partition reciprocal + scalar-mult into a
            q-major O tile shared by the head group; at the last head the O
            tile is transposed back to the d-major OT layout used by WO."""
            nkt0 = 4 * qc + 4
            state = {}
            W1 = HD + 1

            def norm_qt(qtl):
                pob = state["po"]
                qt = 4 * qc + qtl
                if h == 0:
                    o_sbs[(qc, qtl)] = otq.tile([128, 4 * HD], sdt,
                                                tag="otq", name="o_sb")
                o_sb = o_sbs[(qc, qtl)]
                rc = small.tile([128, 1], f32, tag="rc", name="rc")
                nc.vector.reciprocal(
                    rc[:], pob[:, qtl * W1 + HD:qtl * W1 + HD + 1])
                # normalize-and-store (GPSIMD can't read PSUM):
                # out = po * (1/denom) per partition
                nc.vector.tensor_scalar_mul(
                    o_sb[:, h * HD:(h + 1) * HD],
                    pob[:, qtl * W1:qtl * W1 + HD], rc[:])
                if h == HQ - 1:
                    for half in range(2):
                        otr = pw.tile([128, 128], sdt, tag="pv",
                                      name="otr")
                        nc.tensor.transpose(
                            otr[:], o_sb[:, half * 128:half * 128 + 128],
                            ident[:])
                        ots = OT[:, half * S + qt * 128:
                                 half * S + qt * 128 + 128]
                        nc.vector.tensor_copy(ots, otr[:])
                        # hi/lo fp8 for the DoubleRow WO, on idle GPSIMD
                        # (only TensorTensor add/sub are valid Pool opcodes)
                        h8 = OT8h[:, half, qt * 128:qt * 128 + 128]
                        l8 = OT8l[:, half, qt * 128:qt * 128 + 128]
                        nc.gpsimd.tensor_add(h8, ots, zot[:])
                        nc.gpsimd.tensor_sub(l8, ots, h8)
                    del o_sbs[(qc, qtl)]

            def pv_pair(pi):
                if "po" not in state:
                    state["po"] = pjo.tile([128, 512], f32, tag="pjo",
                                           name="pob")
                pob = state["po"]
                for kt, qo, lo, pt_t in tiles[2 * pi:2 * pi + 2]:
                    # qt3's kt-0 matmul goes first with start=True: its
                    # pending-zero covers the whole bank, so the siblings
                    # accumulate onto zeros with start=False (no separate
                    # zero-fill matmul needed)
                    for qtl in (3, 0, 1, 2):
                        if kt > 4 * qc + qtl:
                            continue
                        nc.tensor.matmul(
                            pob[:, qtl * W1:qtl * W1 + W1],
                            pt_t[:, lo + qtl * 128:lo + qtl * 128 + 128],
                            Vp[:, kt * W1:(kt + 1) * W1],
                            start=(kt == 0 and qtl == 3),
                            stop=(kt == 4 * qc + qtl),
                            skip_group_check=True)
                # normalize each q-tile as soon as its group stops (the
                # diagonal k-tile), not all at the end: frees the po bank
                # earlier and shortens the OT critical chain
                if 2 * pi + 2 >= nkt0:
                    for qtl in range(4):
                        norm_qt(qtl)

            return [lambda pi=pi: pv_pair(pi) for pi in range(nkt0 // 2)]

        def wo_half(qt, np2, half, obs, pool=None, ptag="pw",
                    act_copy=False, split_dma=False):
            """One 512-wide n-chunk; the second half fires the paired
            [128,1024] output DMA (or each half its own when split_dma,
            used in the epilogue to shorten the tail)."""
            pool = pool or pw
            if half == 0:
                obs[(qt, np2)] = osb.tile([128, 1024], sdt, tag="ob",
                                          name="ob")
            ob = obs[(qt, np2)]
            ncn = 2 * np2 + half
            pw_t = pool.tile([128, 512], f32, tag=ptag, name="pw_t")
            lhh = OT8h[:, :, qt * 128:qt * 128 + 128]
            llo = OT8l[:, :, qt * 128:qt * 128 + 128]
            rhh = wo_sbh[:, :, ncn * 512:ncn * 512 + 512]
            rlo = wo_sbl[:, :, ncn * 512:ncn * 512 + 512]
            for si, (lt, rt) in enumerate(
                    [(lhh, rhh), (lhh, rlo), (llo, rhh)]):
                nc.tensor.matmul(pw_t[:], lt, rt, start=(si == 0),
                                 stop=(si == 2), perf_mode=DR)
            if act_copy:
                nc.scalar.activation(
                    ob[:, half * 512:half * 512 + 512], pw_t[:],
                    mybir.ActivationFunctionType.Copy, scale=WO_DESCALE)
            else:
                nc.vector.tensor_scalar_mul(
                    ob[:, half * 512:half * 512 + 512], pw_t[:],
                    WO_DESCALE)
            if split_dma:
                nc.sync.dma_start(
                    out[qt * 128:(qt + 1) * 128, ncn * 512:ncn * 512 + 512],
                    ob[:, half * 512:half * 512 + 512])
                if half == 1:
                    del obs[(qt, np2)]
                    wo_copy_flip[0] += 1
            elif half == 1:
                del obs[(qt, np2)]
                wo_copy_flip[0] += 1
                nc.sync.dma_start(
                    out[qt * 128:(qt + 1) * 128,
                        np2 * 1024:np2 * 1024 + 1024], ob[:])
        wo_obs = {}

        # ------------------------------------- merged emission schedule
        def merge(primary, *others):
            """Emit primary thunks; proportionally interleave the others."""
            counters = [0.0] * len(others)
            n = max(1, len(primary))
            for beat in primary:
                for j, lst in enumerate(others):
                    counters[j] += len(lst) / n
                    while counters[j] >= 1.0 and lst:
                        lst.pop(0)()
                        counters[j] -= 1.0
                for th in beat:
                    th()
            for lst in others:
                while lst:
                    lst.pop(0)()

        for th in proj_thunks(0, fused=False):      # prologue (sequential:
            th()                     # KV pass, k-rope, Q passes, V trans)

        prev = None                      # (qc, h, tiles) awaiting PV
        for sc in range(NSC):
            if sc == 1:
                for mt in range(2):
                    nc.sync.dma_start(wo_sbh[:, mt, :],
                                      wo8h[mt * 128:mt * 128 + 128, :])
                    nc.sync.dma_start(wo_sbl[:, mt, :],
                                      wo8l[mt * 128:mt * 128 + 128, :])
            pstream = proj_thunks(sc + 1) if sc + 1 < NSC else []
            if pstream:
                # emit ALL of the next chunk's x-DMA thunks (even k-tiles
                # carry no matmuls) now: the serial DMA queue drains while
                # this chunk computes, so projections never data-starve
                rest = []
                for i, th in enumerate(pstream):
                    if i < KT_D and i % 2 == 0:
                        th()
                    else:
                        rest.append(th)
                pstream = rest
            wostream = ([lambda qt=qt, np2=np2, half=half:
                         wo_half(qt, np2, half, wo_obs)
                         for qt in range(4 * (sc - 1), 4 * (sc - 1) + 4)
                         for np2 in range(2)
                         for half in range(2)] if sc >= 1 else [])
            for h in range(HQ):
                tiles = []
                sth = s_thunks(sc, h, tiles)
                pth = pv_thunks(*prev) if prev is not None else []
                beats = []
                for i in range(max(len(sth), len(pth))):
                    beat = []
                    if i < len(pth):
                        beat.append(pth[i])
                    if i < len(sth):
                        beat.append(sth[i])
                    beats.append(beat)
                # WO of sc-1 needs PV(sc-1, 3) done: that PV is head 0 here
                if h == 0:
                    ptake = max(1, len(pstream) // HQ) if pstream else 0
                    merge(beats, pstream[:ptake])
                    pstream = pstream[ptake:]
                else:
                    ptake = (len(pstream) // (HQ - h)) if pstream else 0
                    wtake = (len(wostream) // (HQ - h)) if wostream else 0
                    merge(beats, pstream[:ptake], wostream[:wtake])
                    pstream = pstream[ptake:]
                    wostream = wostream[wtake:]
                prev = (sc, h, tiles)
            merge([], pstream, wostream)

        # epilogue: PV of the last head, then WO of chunk 3; the score
        # pool's banks are free now, so WO rotates through those too
        for th in pv_thunks(*prev):
            th()
        epi = 0
        pools = [(pw, "pw"), (ps, "ps"), (pjo, "pjo")]
        for qt in range(12, 16):
            for np2 in range(2):
                pool, ptag = pools[epi % 3]
                for half in range(2):
                    wo_half(qt, np2, half, wo_obs, pool=pool, ptag=ptag,
                            act_copy=(epi % 2 == 0), split_dma=True)
                epi += 1

    nc.compile()
    return nc


# ------------------------------------------------------------- host side
def _pair_perm64():
    """Column permutation putting the RoPE partner 16 partitions away."""
    return np.array([2 * (16 * (j // 32) + (j % 16)) + ((j % 32) // 16)
                     for j in range(64)])


def _split8(a, scale):
    """Split a*scale into fp8e4m3 hi + lo at the same scale."""
    hi = np.asarray(a * scale).astype(ml_dtypes.float8_e4m3)
    lo = (a * scale - hi.astype(np.float32)).astype(ml_dtypes.float8_e4m3)
    return np.ascontiguousarray(hi), np.ascontiguousarray(lo)


def _host_prep(x, freqs_cos, freqs_sin, wq, wk, wv, wo):
    _, _, npdt = _dtypes()
    x = np.asarray(x, np.float32)
    fc = np.asarray(freqs_cos, np.float32)
    fs = np.asarray(freqs_sin, np.float32)
    wq = np.asarray(wq, np.float32)
    wk = np.asarray(wk, np.float32)
    wv = np.asarray(wv, np.float32)
    wo = np.asarray(wo, np.float32)

    perm = _pair_perm64()
    x8h, x8l = _split8(np.ascontiguousarray(x[0].T), SXH)

    p = np.arange(64)
    pair = 16 * ((p % 64) // 32) + (p % 16)
    sign = np.where((p % 32) < 16, -1.0, 1.0).astype(np.float32)
    cosE = np.ascontiguousarray(fc[:, pair].T)                  # [64, S]
    sinE = np.ascontiguousarray(fs[:, pair].T) * sign[:, None]  # [64, S]
    utri = np.triu(np.ones((128, 128), np.float32)).astype(npdt)

    in_maps = []
    for c in range(NCORES):
        qcols = np.concatenate(
            [wq[:, (4 * c + i) * 64 + perm] for i in range(HQ)], axis=1)
        kcols = wk[:, c * 64 + perm]
        vcols = wv[:, c * 64:(c + 1) * 64]
        wqkv_c = np.concatenate([qcols, kcols, vcols], axis=1)
        w8h, w8l = _split8(wqkv_c, SWH)
        wo8h_c, wo8l_c = _split8(wo[QW * c:QW * (c + 1), :], SWO)
        in_maps.append({
            "x8h": x8h,
            "x8l": x8l,
            "w8h": w8h,
            "w8l": w8l,
            "wo8h": wo8h_c,
            "wo8l": wo8l_c,
            "cosE": cosE.astype(npdt),
            "sinE": np.ascontiguousarray(sinE).astype(npdt),
            "utri": np.ascontiguousarray(utri),
        })
    return in_maps


_NC_CACHE = {}


def get_program():
    if "nc" not in _NC_CACHE:
        _NC_CACHE["nc"] = build_program()
    return _NC_CACHE["nc"]


def kernel(x, freqs_cos, freqs_sin, wq, wk, wv, wo):
    nc = get_program()
    in_maps = _host_prep(x, freqs_cos, freqs_sin, wq, wk, wv, wo)
    res = run_bass_kernel_spmd(nc, in_maps, core_ids=list(range(NCORES)))
    acc = np.zeros((S, DIM), np.float64)
    for r in res.results:
        acc += r["out"].astype(np.float64)
    return acc.astype(np.float32).reshape(1, S, DIM)

